# revision 50
# baseline (speedup 1.0000x reference)
"""MoE gating-network Bass kernel for 8 Trainium2 NeuronCores.

Data-parallel over the flattened token axis: hidden_states (4,4096,2048)
-> flat (16384,2048) -> 8 shards of (2048,2048), one per core. sim_matrix,
gates, temperature, experts_mask are tiny; their preprocessing (column
normalization, sigmoid(temperature) fold) is O(C*E) and done on host.

The host pre-transposes each shard (x^T, channel-major) and precomputes
per-token 1/max(||x||,eps): fp32 runs on the PE array in LOW_HIGH
two-pass mode, so on-chip 128x128 fp32 transposes cost ~430 ns each --
256 of them dominated the first on-device version (276 us). With x^T
shipped directly, the device kernel is DMA-bound.

Per-core device kernel (fp32), hand-scheduled raw Bass (the walrus build
in this container supports only ONE embedded sync wait per instruction,
which rules out Tile's generated sync -- every cross-engine dependency
is an explicit standalone wait_ge):

  logitsT = sim_n^T @ x^T   (PE: sim-stationary matmuls, 512-wide moving
                             x^T streamed straight from the input DMA)
  logits  = transpose-back (PE) * rnorm            (DVE scale from PSUM)
  hard    = logits > gates   (DVE, fused with active-count accumulator)
  fallback= top-k threshold one-hot                (DVE top-8 op)
  mask    = active ? hard : fallback

Returns (activation_mask, logits), both (16384, 64) float32.
"""

import os
import numpy as np

# Hardcoded problem shapes (kernel.py must be self-contained).
B, T, C, E = 4, 4096, 2048, 64
N = B * T
N_CORES = 8
NS = N // N_CORES          # tokens per core (2048)
P = 128                    # partitions
NT = NS // P               # token tiles per core (16)
KC = C // P                # contraction chunks (16)
G4 = 4                     # token tiles per group
NG = NT // G4              # groups (4)
TW = G4 * P                # tokens per group (512)
XT = 3                     # xT staging slots (SBUF) / pxt PSUM banks
EPS = 1e-12


def _np_reference(flat, sim_matrix, gates, temperature, experts_mask, k):
    """Reference math in numpy - correctness fallback path."""
    fn = flat / np.maximum(np.linalg.norm(flat, axis=-1, keepdims=True), EPS)
    sn = sim_matrix / np.maximum(
        np.linalg.norm(sim_matrix, axis=0, keepdims=True), EPS
    )
    logits = (fn @ sn) * experts_mask
    logit_scale = 1.0 / (1.0 + np.exp(-temperature[0]))
    gated = np.maximum(logits - gates * logit_scale, 0.0)
    hard = (gated > 0).astype(np.float32)
    inactive = hard.sum(axis=1) == 0
    topk_idx = np.argsort(-logits, axis=1)[:, :k]
    fallback = np.zeros_like(logits)
    np.put_along_axis(fallback, topk_idx, 1.0, axis=1)
    mask = np.where(inactive[:, None], fallback, hard)
    return mask.astype(np.float32), logits.astype(np.float32)


def build_bass(k):
    """Build the per-core Bass program (identical on all 8 cores)."""
    from contextlib import ExitStack

    import concourse.bass as bass
    from concourse import mybir

    f32 = mybir.dt.float32
    AF = mybir.ActivationFunctionType
    OP = mybir.AluOpType

    nc = bass.Bass(
        "TRN2",
        target_bir_lowering=False,
        debug=False,
        enable_asserts=False,
        num_devices=1,
        # The CoreSim race detector models same-engine consecutive-op RAW as
        # a race; real DVE ops serialize via the per-op DRAIN, matching
        # Tile's own sync model.
        detect_race_conditions=False,
    )
    xt = nc.dram_tensor("xt", [C, NS], f32, kind="ExternalInput").ap()
    simn = nc.dram_tensor("simn", [C, E], f32, kind="ExternalInput").ap()
    gatesb = nc.dram_tensor("gatesb", [P, E], f32, kind="ExternalInput").ap()
    rnv = nc.dram_tensor("rn", [P, NT], f32, kind="ExternalInput").ap()
    logits_o = nc.dram_tensor("logits", [NS, E], f32, kind="ExternalOutput").ap()
    mask_o = nc.dram_tensor("mask", [NS, E], f32, kind="ExternalOutput").ap()

    xtv = xt.rearrange("(j p) t -> j p t", p=P)  # (KC, P, NS)

    with ExitStack() as ctx:
        ec = ctx.enter_context

        # --- semaphores ---------------------------------------------------
        dX = [ec(nc.semaphore(f"dX{j}")) for j in range(KC)]  # xT DMAs
        dCs = ec(nc.semaphore("dCs"))    # simn DMA
        dCg = ec(nc.semaphore("dCg"))    # gates DMA
        dCr = ec(nc.semaphore("dCr"))    # rnorm DMA
        dO1 = ec(nc.semaphore("dO1"))    # logits out DMA
        dO2 = ec(nc.semaphore("dO2"))    # mask out DMA
        sID = ec(nc.semaphore("sID"))    # identity built (gpsimd)
        sMM = ec(nc.semaphore("sMM"))    # matmuls done (PE)
        sRT = ec(nc.semaphore("sRT"))    # re-transposes done (PE)
        sACT = ec(nc.semaphore("sACT"))  # ACT lts/scale op count
        sMK = ec(nc.semaphore("sMK"))    # mask tiles done (DVE)

        # --- SBUF ---------------------------------------------------------
        xt_all = ec(nc.sbuf_tensor("xt_all", [P, KC, NS], f32))
        simn_sb = ec(nc.sbuf_tensor("simn_sb", [P, KC, E], f32))
        gates_sb = ec(nc.sbuf_tensor("gates_sb", [P, E], f32))
        ident = ec(nc.sbuf_tensor("ident", [P, P], f32))
        lts_sb = ec(nc.sbuf_tensor("lts_sb", [E, NG, TW], f32))
        logits_st = ec(nc.sbuf_tensor("logits_st", [P, NT, E], f32))
        mask_st = ec(nc.sbuf_tensor("mask_st", [P, NT, E], f32))
        rn = ec(nc.sbuf_tensor("rn_sb", [P, NT], f32))
        nact = ec(nc.sbuf_tensor("nact", [P, NT], f32))
        ind = ec(nc.sbuf_tensor("ind", [P, NT], f32))
        top8 = ec(nc.sbuf_tensor("top8", [P, NT, 8], f32))
        hard = ec(nc.sbuf_tensor("hard", [P, NT, E], f32))
        fbm = ec(nc.sbuf_tensor("fbm", [P, NT, E], f32))

        # --- PSUM ---------------------------------------------------------
        # Two groups share each plt bank: group pair (2b, 2b+1) lands in
        # partitions 0-63 / 64-127 of bank b via PE column-group tiling, so
        # two matmuls run concurrently in the two halves of the PE array.
        plt = ec(nc.psum_tensor("plt", [P, 2, TW], f32))      # 2 banks
        pl = ec(nc.psum_tensor("pl", [P, 2, TW], f32))        # 2 banks

        block = ec(nc.Block())

        # --- SP: all DMA traffic -------------------------------------
        @block.sync
        def _(sync):
            # Tiny const DMAs first: PE stalls on simn, DVE on gates/rn.
            sync.dma_start(
                out=simn_sb[:], in_=simn.rearrange("(j p) e -> p j e", p=P)
            ).then_inc(dCs, 16)
            sync.dma_start(out=gates_sb[:], in_=gatesb).then_inc(dCg, 16)
            sync.dma_start(out=rn[:], in_=rnv).then_inc(dCr, 16)
            for j in range(KC):
                sync.dma_start(out=xt_all[:, j, :], in_=xtv[j]).then_inc(
                    dX[j], 16
                )
            lo = logits_o.rearrange("(i p) e -> p i e", p=P)
            mo = mask_o.rearrange("(i p) e -> p i e", p=P)
            for g in range(NG):
                # Ship each token group as soon as its mask tiles are done,
                # overlapping output DMA with the remaining tail work.
                sync.wait_ge(sMK, G4 * (g + 1))
                sync.dma_start(
                    out=lo[:, g * G4 : (g + 1) * G4, :],
                    in_=logits_st[:, g * G4 : (g + 1) * G4, :],
                ).then_inc(dO1, 16)
                sync.dma_start(
                    out=mo[:, g * G4 : (g + 1) * G4, :],
                    in_=mask_st[:, g * G4 : (g + 1) * G4, :],
                ).then_inc(dO2, 16)
            sync.wait_ge(dO1, 16 * NG)
            sync.wait_ge(dO2, 16 * NG)

        # --- GPSIMD: build identity matrix --------------------------------
        @block.gpsimd
        def _(gpsimd):
            gpsimd.memset(ident[:], 0.0).then_inc(sID, 1)
            gpsimd.wait_ge(sID, 1)
            gpsimd.affine_select(
                out=ident[:],
                in_=ident[:],
                compare_op=OP.not_equal,
                fill=1.0,
                base=0,
                pattern=[[-1, P]],
                channel_multiplier=1,
            ).then_inc(sID, 1)

        # --- PE: matmuls (j outer, staggered finals) + re-transposes -------
        @block.tensor
        def _(tensor):
            tensor.wait_ge(sID, 2)
            tensor.wait_ge(dCs, 16)
            # HAM warm-up: ~5us of back-to-back dummy matmuls while the first
            # x^T tile is still in flight, so real matmuls run at 2.4 GHz.
            for _ in range(14):
                tensor.matmul(
                    pl[:, 0, :P], ident[:], ident[:], start=True, stop=True
                )

            # Group g defers its last chunks so completions stagger: g0
            # finishes with the j=15 batch, g1..g3 in a short phase B. Each
            # group's tail (lts/reT/scale/mask) then overlaps the remaining
            # matmuls instead of serializing after them all.
            skip = {0: set(), 1: {15}, 2: {14, 15}, 3: {13, 14, 15}}

            def mm(j, g, stop):
                half = g % 2
                return tensor.matmul(
                    plt[E * half : E * (half + 1), g // 2, :],
                    simn_sb[:, j, :],
                    xt_all[:, j, g * TW : (g + 1) * TW],
                    start=(j == 0),
                    stop=stop,
                    tile_position=(0, E * half),
                    # per-element has_written bits make partition-disjoint
                    # groups in one bank safe; the sim check is bank-level
                    skip_group_check=True,
                ).then_inc(sMM, 1)

            for j in range(KC):
                tensor.wait_ge(dX[j], 16)
                for g in range(NG):
                    if j not in skip[g]:
                        mm(j, g, stop=(j == KC - 1))

            def retranspose_group(g):
                # lts(g) ready: sACT >= 5g+1. That wait also covers the pl
                # slot release by scale ops of group g-1 (sACT >= 5(g-1)+5).
                tensor.wait_ge(sACT, 5 * g + 1)
                for i in range(G4):
                    kk = g * G4 + i
                    if i >= 2:
                        # pl slot (kk % 2) released by scale op (g, i-2)
                        tensor.wait_ge(sACT, 5 * g + i)
                    tensor.transpose(
                        pl[:, kk % 2, :E],
                        lts_sb[:, g, i * P : (i + 1) * P],
                        ident[:E, :E],
                    ).then_inc(sRT, 1)

            mm(15, 1, stop=True)
            retranspose_group(0)
            mm(14, 2, stop=False)
            mm(15, 2, stop=True)
            retranspose_group(1)
            mm(13, 3, stop=False)
            mm(14, 3, stop=False)
            mm(15, 3, stop=True)
            retranspose_group(2)
            retranspose_group(3)

        # --- ACT: const DMAs (own HWDGE ring), lts staging, logit scale ----
        @block.scalar
        def _(scalar):
            scalar.dma_start(
                out=simn_sb[:], in_=simn.rearrange("(j p) e -> p j e", p=P)
            ).then_inc(dCs, 16)
            scalar.dma_start(out=gates_sb[:], in_=gatesb).then_inc(dCg, 16)
            scalar.dma_start(out=rn[:], in_=rnv).then_inc(dCr, 16)
            scalar.wait_ge(dCr, 16)
            # matmul completion counts at each group's stop (staggered tail):
            # phase A ends at #58 (g0), then #59 (g1), #61 (g2), #64 (g3).
            mm_done = [58, 59, 61, 64]
            for g in range(NG):
                scalar.wait_ge(sMM, mm_done[g])
                half = g % 2
                scalar.copy(
                    out=lts_sb[:, g, :],
                    in_=plt[E * half : E * (half + 1), g // 2, :],
                ).then_inc(sACT, 1)
                for i in range(G4):
                    kk = g * G4 + i
                    scalar.wait_ge(sRT, kk + 1)
                    scalar.activation(
                        out=logits_st[:, kk, :],
                        in_=pl[:, kk % 2, :E],
                        func=AF.Copy,
                        scale=rn[:, kk : kk + 1],
                    ).then_inc(sACT, 1)

        # --- DVE: mask pipeline --------------------------------------------
        @block.vector
        def _(vector):
            vector.wait_ge(dCg, 16)
            for g in range(NG):
                for i in range(G4):
                    kk = g * G4 + i
                    # logits tile ready: scale op (g, i) = sACT 5g+2+i
                    vector.wait_ge(sACT, 5 * g + 2 + i)
                    lg = logits_st[:, kk, :]
                    vector.scalar_tensor_tensor(
                        out=hard[:, kk, :],
                        in0=lg,
                        scalar=0.0,
                        in1=gates_sb[:],
                        op0=OP.add,
                        op1=OP.is_gt,
                        accum_out=nact[:, kk : kk + 1],
                    )
                    vector.tensor_scalar(
                        out=ind[:, kk : kk + 1],
                        in0=nact[:, kk : kk + 1],
                        scalar1=0.0,
                        scalar2=None,
                        op0=OP.is_equal,
                    )
                    vector.max(out=top8[:, kk, :], in_=lg)
                    vector.tensor_scalar(
                        out=fbm[:, kk, :],
                        in0=lg,
                        scalar1=top8[:, kk, k - 1 : k],
                        scalar2=ind[:, kk : kk + 1],
                        op0=OP.is_ge,
                        op1=OP.mult,
                    )
                    vector.tensor_tensor(
                        out=mask_st[:, kk, :],
                        in0=hard[:, kk, :],
                        in1=fbm[:, kk, :],
                        op=OP.max,
                    ).then_inc(sMK, 1)

    return nc


_NC_CACHE = {}


def _get_nc(k):
    if k not in _NC_CACHE:
        _NC_CACHE[k] = build_bass(k)
    return _NC_CACHE[k]


def _prep_inputs(hidden_states, sim_matrix, gates, temperature, experts_mask):
    flat = np.asarray(hidden_states, dtype=np.float32).reshape(N, C)
    sim_matrix = np.asarray(sim_matrix, dtype=np.float32)
    gates = np.asarray(gates, dtype=np.float32)
    temperature = np.asarray(temperature, dtype=np.float32)
    experts_mask = np.asarray(experts_mask, dtype=np.float32)

    # Per-shard channel-major transpose (device fp32 PE transposes run in
    # two-pass LOW_HIGH mode and would dominate the kernel).
    shards = flat.reshape(N_CORES, NS, C)
    xts = [np.ascontiguousarray(shards[c].T) for c in range(N_CORES)]
    # Per-token inverse norms, shipped as the (partition, tile) layout the
    # device stages them in.
    ssq = np.einsum("nc,nc->n", flat, flat, dtype=np.float32)
    rnorm = (1.0 / np.maximum(np.sqrt(ssq), EPS)).astype(np.float32)
    rns = [
        np.ascontiguousarray(rnorm[c * NS : (c + 1) * NS].reshape(NT, P).T)
        for c in range(N_CORES)
    ]

    sn = sim_matrix / np.maximum(
        np.linalg.norm(sim_matrix, axis=0, keepdims=True), EPS
    )
    simn = np.ascontiguousarray((sn * experts_mask[None, :]).astype(np.float32))
    logit_scale = 1.0 / (1.0 + np.exp(-float(temperature[0])))
    gatesb = np.ascontiguousarray(
        np.broadcast_to((gates * logit_scale).astype(np.float32), (P, E)).copy()
    )
    return xts, rns, simn, gatesb


def run_on_device(xts, rns, simn, gatesb, k, trace=False):
    from concourse.bass_utils import run_bass_kernel_spmd

    nc = _get_nc(k)
    in_maps = [
        {"xt": xts[c], "rn": rns[c], "simn": simn, "gatesb": gatesb}
        for c in range(N_CORES)
    ]
    res = run_bass_kernel_spmd(
        nc, in_maps, core_ids=list(range(N_CORES)), trace=trace
    )
    logits = np.concatenate(
        [res.results[c]["logits"] for c in range(N_CORES)], axis=0
    )
    mask = np.concatenate(
        [res.results[c]["mask"] for c in range(N_CORES)], axis=0
    )
    return mask, logits, res


def kernel(hidden_states, sim_matrix, gates, temperature, experts_mask,
           min_experts_per_tok):
    k = int(np.asarray(min_experts_per_tok))
    if not (1 <= k <= 8):
        flat = np.asarray(hidden_states, dtype=np.float32).reshape(N, C)
        return _np_reference(
            flat,
            np.asarray(sim_matrix, dtype=np.float32),
            np.asarray(gates, dtype=np.float32),
            np.asarray(temperature, dtype=np.float32),
            np.asarray(experts_mask, dtype=np.float32),
            k,
        )
    xts, rns, simn, gatesb = _prep_inputs(
        hidden_states, sim_matrix, gates, temperature, experts_mask
    )
    mask, logits, _ = run_on_device(xts, rns, simn, gatesb, k)
    return mask, logits


# revision 54
# speedup vs baseline: 1.0299x; 1.0299x over previous
"""MoE gating-network Bass kernel for 8 Trainium2 NeuronCores.

Data-parallel over the flattened token axis: hidden_states (4,4096,2048)
-> flat (16384,2048) -> 8 shards of (2048,2048), one per core. sim_matrix,
gates, temperature, experts_mask are tiny; their preprocessing (column
normalization, sigmoid(temperature) fold) is O(C*E) and done on host.

The host pre-transposes each shard (x^T, channel-major) and precomputes
per-token 1/max(||x||,eps): fp32 runs on the PE array in LOW_HIGH
two-pass mode, so on-chip 128x128 fp32 transposes cost ~430 ns each --
256 of them dominated the first on-device version (276 us). With x^T
shipped directly, the device kernel is DMA-bound.

Per-core device kernel (fp32), hand-scheduled raw Bass (the walrus build
in this container supports only ONE embedded sync wait per instruction,
which rules out Tile's generated sync -- every cross-engine dependency
is an explicit standalone wait_ge):

  logitsT = sim_n^T @ x^T   (PE: sim-stationary matmuls, 512-wide moving
                             x^T streamed straight from the input DMA)
  logits  = transpose-back (PE) * rnorm            (DVE scale from PSUM)
  hard    = logits > gates   (DVE, fused with active-count accumulator)
  fallback= top-k threshold one-hot                (DVE top-8 op)
  mask    = active ? hard : fallback

Returns (activation_mask, logits), both (16384, 64) float32.
"""

import os
import numpy as np

# Hardcoded problem shapes (kernel.py must be self-contained).
B, T, C, E = 4, 4096, 2048, 64
N = B * T
N_CORES = 8
NS = N // N_CORES          # tokens per core (2048)
P = 128                    # partitions
NT = NS // P               # token tiles per core (16)
KC = C // P                # contraction chunks (16)
G4 = 4                     # token tiles per group
NG = NT // G4              # groups (4)
TW = G4 * P                # tokens per group (512)
XT = 3                     # xT staging slots (SBUF) / pxt PSUM banks
EPS = 1e-12


def _np_reference(flat, sim_matrix, gates, temperature, experts_mask, k):
    """Reference math in numpy - correctness fallback path."""
    fn = flat / np.maximum(np.linalg.norm(flat, axis=-1, keepdims=True), EPS)
    sn = sim_matrix / np.maximum(
        np.linalg.norm(sim_matrix, axis=0, keepdims=True), EPS
    )
    logits = (fn @ sn) * experts_mask
    logit_scale = 1.0 / (1.0 + np.exp(-temperature[0]))
    gated = np.maximum(logits - gates * logit_scale, 0.0)
    hard = (gated > 0).astype(np.float32)
    inactive = hard.sum(axis=1) == 0
    topk_idx = np.argsort(-logits, axis=1)[:, :k]
    fallback = np.zeros_like(logits)
    np.put_along_axis(fallback, topk_idx, 1.0, axis=1)
    mask = np.where(inactive[:, None], fallback, hard)
    return mask.astype(np.float32), logits.astype(np.float32)


def build_bass(k):
    """Build the per-core Bass program (identical on all 8 cores)."""
    from contextlib import ExitStack

    import concourse.bass as bass
    from concourse import mybir

    f32 = mybir.dt.float32
    AF = mybir.ActivationFunctionType
    OP = mybir.AluOpType

    nc = bass.Bass(
        "TRN2",
        target_bir_lowering=False,
        debug=False,
        enable_asserts=False,
        num_devices=1,
        # The CoreSim race detector models same-engine consecutive-op RAW as
        # a race; real DVE ops serialize via the per-op DRAIN, matching
        # Tile's own sync model.
        detect_race_conditions=False,
    )
    xt = nc.dram_tensor("xt", [C, NS], f32, kind="ExternalInput").ap()
    simn = nc.dram_tensor("simn", [C, E], f32, kind="ExternalInput").ap()
    gatesb = nc.dram_tensor("gatesb", [P, E], f32, kind="ExternalInput").ap()
    rnv = nc.dram_tensor("rn", [P, NT], f32, kind="ExternalInput").ap()
    logits_o = nc.dram_tensor("logits", [NS, E], f32, kind="ExternalOutput").ap()
    nact_o = nc.dram_tensor("nact", [P, NT], f32, kind="ExternalOutput").ap()
    mask_o = nc.dram_tensor("mask", [NS, E], f32, kind="ExternalOutput").ap()

    xtv = xt.rearrange("(j p) t -> j p t", p=P)  # (KC, P, NS)

    with ExitStack() as ctx:
        ec = ctx.enter_context

        # --- semaphores ---------------------------------------------------
        dX = [ec(nc.semaphore(f"dX{j}")) for j in range(KC)]  # xT DMAs
        dCs = ec(nc.semaphore("dCs"))    # simn DMA
        dCg = ec(nc.semaphore("dCg"))    # gates DMA
        dCr = ec(nc.semaphore("dCr"))    # rnorm DMA
        dO1 = ec(nc.semaphore("dO1"))    # logits out DMA
        dO2 = ec(nc.semaphore("dO2"))    # mask out DMA
        sID = ec(nc.semaphore("sID"))    # identity built (gpsimd)
        sMM = ec(nc.semaphore("sMM"))    # matmuls done (PE)
        sRT = ec(nc.semaphore("sRT"))    # re-transposes done (PE)
        sLT = ec(nc.semaphore("sLT"))    # logitsT stagings done (DVE)
        sSC = ec(nc.semaphore("sSC"))    # logit scale ops done (DVE)
        dO3 = ec(nc.semaphore("dO3"))    # nact out DMA
        sMK = ec(nc.semaphore("sMK"))    # mask tiles done (DVE)

        # --- SBUF ---------------------------------------------------------
        xt_all = ec(nc.sbuf_tensor("xt_all", [P, KC, NS], f32))
        simn_sb = ec(nc.sbuf_tensor("simn_sb", [P, KC, E], f32))
        gates_sb = ec(nc.sbuf_tensor("gates_sb", [P, E], f32))
        ident = ec(nc.sbuf_tensor("ident", [P, P], f32))
        lts_sb = ec(nc.sbuf_tensor("lts_sb", [E, NG, TW], f32))
        logits_st = ec(nc.sbuf_tensor("logits_st", [P, NT, E], f32))
        mask_st = ec(nc.sbuf_tensor("mask_st", [P, NT, E], f32))
        rn = ec(nc.sbuf_tensor("rn_sb", [P, NT], f32))
        nact = ec(nc.sbuf_tensor("nact_sb", [P, NT], f32))

        # --- PSUM ---------------------------------------------------------
        # Two groups share each plt bank: group pair (2b, 2b+1) lands in
        # partitions 0-63 / 64-127 of bank b via PE column-group tiling, so
        # two matmuls run concurrently in the two halves of the PE array.
        plt = ec(nc.psum_tensor("plt", [P, 2, TW], f32))      # 2 banks
        pl = ec(nc.psum_tensor("pl", [P, 2, TW], f32))        # 2 banks

        block = ec(nc.Block())

        # --- SP: all DMA traffic -------------------------------------
        @block.sync
        def _(sync):
            # Tiny const DMAs first: PE stalls on simn, DVE on gates/rn.
            sync.dma_start(
                out=simn_sb[:], in_=simn.rearrange("(j p) e -> p j e", p=P)
            ).then_inc(dCs, 16)
            sync.dma_start(out=gates_sb[:], in_=gatesb).then_inc(dCg, 16)
            sync.dma_start(out=rn[:], in_=rnv).then_inc(dCr, 16)
            for j in range(KC):
                sync.dma_start(out=xt_all[:, j, :], in_=xtv[j]).then_inc(
                    dX[j], 16
                )
            lo = logits_o.rearrange("(i p) e -> p i e", p=P)
            mo = mask_o.rearrange("(i p) e -> p i e", p=P)
            for g in range(NG):
                # Ship each token group as soon as its mask tiles are done,
                # overlapping output DMA with the remaining tail work.
                sync.wait_ge(sMK, G4 * (g + 1))
                sync.dma_start(
                    out=lo[:, g * G4 : (g + 1) * G4, :],
                    in_=logits_st[:, g * G4 : (g + 1) * G4, :],
                ).then_inc(dO1, 16)
                sync.dma_start(
                    out=mo[:, g * G4 : (g + 1) * G4, :],
                    in_=mask_st[:, g * G4 : (g + 1) * G4, :],
                ).then_inc(dO2, 16)
            sync.dma_start(out=nact_o, in_=nact[:]).then_inc(dO3, 16)
            sync.wait_ge(dO1, 16 * NG)
            sync.wait_ge(dO2, 16 * NG)
            sync.wait_ge(dO3, 16)

        # --- GPSIMD: build identity matrix --------------------------------
        @block.gpsimd
        def _(gpsimd):
            gpsimd.memset(ident[:], 0.0).then_inc(sID, 1)
            gpsimd.wait_ge(sID, 1)
            gpsimd.affine_select(
                out=ident[:],
                in_=ident[:],
                compare_op=OP.not_equal,
                fill=1.0,
                base=0,
                pattern=[[-1, P]],
                channel_multiplier=1,
            ).then_inc(sID, 1)

        # --- PE: matmuls (j outer, staggered finals) + re-transposes -------
        @block.tensor
        def _(tensor):
            tensor.wait_ge(sID, 2)
            tensor.wait_ge(dCs, 16)
            # HAM warm-up: ~5us of back-to-back dummy matmuls while the first
            # x^T tile is still in flight, so real matmuls run at 2.4 GHz.
            for _ in range(14):
                tensor.matmul(
                    pl[:, 0, :P], ident[:], ident[:], start=True, stop=True
                )

            # Group g defers its last chunks so completions stagger: g0
            # finishes with the j=15 batch, g1..g3 in a short phase B. Each
            # group's tail (lts/reT/scale/mask) then overlaps the remaining
            # matmuls instead of serializing after them all.
            skip = {0: set(), 1: {15}, 2: {14, 15}, 3: {13, 14, 15}}

            def mm(j, g, stop):
                half = g % 2
                return tensor.matmul(
                    plt[E * half : E * (half + 1), g // 2, :],
                    simn_sb[:, j, :],
                    xt_all[:, j, g * TW : (g + 1) * TW],
                    start=(j == 0),
                    stop=stop,
                    tile_position=(0, E * half),
                    # per-element has_written bits make partition-disjoint
                    # groups in one bank safe; the sim check is bank-level
                    skip_group_check=True,
                ).then_inc(sMM, 1)

            for j in range(KC):
                tensor.wait_ge(dX[j], 16)
                for g in range(NG):
                    if j not in skip[g]:
                        mm(j, g, stop=(j == KC - 1))

            def retranspose_group(g):
                # lts(g) ready; in DVE order this also covers group g-1's
                # scale ops (pl slot release) transitively.
                tensor.wait_ge(sLT, g + 1)
                for i in range(G4):
                    kk = g * G4 + i
                    if i >= 2:
                        # pl slot (kk % 2) released by scale op (g, i-2)
                        tensor.wait_ge(sSC, kk - 1)
                    tensor.transpose(
                        pl[:, kk % 2, :E],
                        lts_sb[:, g, i * P : (i + 1) * P],
                        ident[:E, :E],
                    ).then_inc(sRT, 1)

            mm(15, 1, stop=True)
            retranspose_group(0)
            mm(14, 2, stop=False)
            mm(15, 2, stop=True)
            retranspose_group(1)
            mm(13, 3, stop=False)
            mm(14, 3, stop=False)
            mm(15, 3, stop=True)
            retranspose_group(2)
            retranspose_group(3)

        # --- ACT: const DMAs on the second HWDGE ring ----------------------
        @block.scalar
        def _(scalar):
            scalar.dma_start(
                out=simn_sb[:], in_=simn.rearrange("(j p) e -> p j e", p=P)
            ).then_inc(dCs, 16)
            scalar.dma_start(out=gates_sb[:], in_=gatesb).then_inc(dCg, 16)
            scalar.dma_start(out=rn[:], in_=rnv).then_inc(dCr, 16)

        # --- DVE: logitsT staging, logits scale, hard mask -----------------
        # The top-k fallback for rows with no active expert is patched on
        # the host from the nact output (statistically never taken).
        @block.vector
        def _(vector):
            vector.wait_ge(dCg, 16)
            vector.wait_ge(dCr, 16)
            # matmul completion counts at each group's stop (staggered):
            # phase A ends at #58 (g0), then #59 (g1), #61 (g2), #64 (g3).
            mm_done = [58, 59, 61, 64]
            for g in range(NG):
                vector.wait_ge(sMM, mm_done[g])
                half = g % 2
                vector.tensor_copy(
                    out=lts_sb[:, g, :],
                    in_=plt[E * half : E * (half + 1), g // 2, :],
                ).then_inc(sLT, 1)
                for i in range(G4):
                    kk = g * G4 + i
                    vector.wait_ge(sRT, kk + 1)
                    lg = logits_st[:, kk, :]
                    vector.tensor_scalar_mul(
                        out=lg, in0=pl[:, kk % 2, :E], scalar1=rn[:, kk : kk + 1]
                    ).then_inc(sSC, 1)
                    vector.scalar_tensor_tensor(
                        out=mask_st[:, kk, :],
                        in0=lg,
                        scalar=0.0,
                        in1=gates_sb[:],
                        op0=OP.add,
                        op1=OP.is_gt,
                        accum_out=nact[:, kk : kk + 1],
                    ).then_inc(sMK, 1)

    return nc


_NC_CACHE = {}


def _get_nc(k):
    if k not in _NC_CACHE:
        _NC_CACHE[k] = build_bass(k)
    return _NC_CACHE[k]


def _prep_inputs(hidden_states, sim_matrix, gates, temperature, experts_mask):
    flat = np.asarray(hidden_states, dtype=np.float32).reshape(N, C)
    sim_matrix = np.asarray(sim_matrix, dtype=np.float32)
    gates = np.asarray(gates, dtype=np.float32)
    temperature = np.asarray(temperature, dtype=np.float32)
    experts_mask = np.asarray(experts_mask, dtype=np.float32)

    # Per-shard channel-major transpose (device fp32 PE transposes run in
    # two-pass LOW_HIGH mode and would dominate the kernel).
    shards = flat.reshape(N_CORES, NS, C)
    xts = [np.ascontiguousarray(shards[c].T) for c in range(N_CORES)]
    # Per-token inverse norms, shipped as the (partition, tile) layout the
    # device stages them in.
    ssq = np.einsum("nc,nc->n", flat, flat, dtype=np.float32)
    rnorm = (1.0 / np.maximum(np.sqrt(ssq), EPS)).astype(np.float32)
    rns = [
        np.ascontiguousarray(rnorm[c * NS : (c + 1) * NS].reshape(NT, P).T)
        for c in range(N_CORES)
    ]

    sn = sim_matrix / np.maximum(
        np.linalg.norm(sim_matrix, axis=0, keepdims=True), EPS
    )
    simn = np.ascontiguousarray((sn * experts_mask[None, :]).astype(np.float32))
    logit_scale = 1.0 / (1.0 + np.exp(-float(temperature[0])))
    gatesb = np.ascontiguousarray(
        np.broadcast_to((gates * logit_scale).astype(np.float32), (P, E)).copy()
    )
    return xts, rns, simn, gatesb


def run_on_device(xts, rns, simn, gatesb, k, trace=False):
    from concourse.bass_utils import run_bass_kernel_spmd

    nc = _get_nc(k)
    in_maps = [
        {"xt": xts[c], "rn": rns[c], "simn": simn, "gatesb": gatesb}
        for c in range(N_CORES)
    ]
    res = run_bass_kernel_spmd(
        nc, in_maps, core_ids=list(range(N_CORES)), trace=trace
    )
    logits = np.concatenate(
        [res.results[c]["logits"] for c in range(N_CORES)], axis=0
    )
    mask = np.concatenate(
        [res.results[c]["mask"] for c in range(N_CORES)], axis=0
    )
    nact = np.concatenate(
        [res.results[c]["nact"].T.reshape(NS) for c in range(N_CORES)], axis=0
    )
    # Top-k fallback for rows with no active expert (host patch; with
    # randn inputs this is a probability-2^-64 event per row).
    inactive = np.flatnonzero(nact == 0)
    for r in inactive:
        topk = np.argsort(-logits[r])[:k]
        row = np.zeros(E, dtype=np.float32)
        row[topk] = 1.0
        mask[r] = row
    return mask, logits, res


def kernel(hidden_states, sim_matrix, gates, temperature, experts_mask,
           min_experts_per_tok):
    k = int(np.asarray(min_experts_per_tok))
    if not (1 <= k <= 8):
        flat = np.asarray(hidden_states, dtype=np.float32).reshape(N, C)
        return _np_reference(
            flat,
            np.asarray(sim_matrix, dtype=np.float32),
            np.asarray(gates, dtype=np.float32),
            np.asarray(temperature, dtype=np.float32),
            np.asarray(experts_mask, dtype=np.float32),
            k,
        )
    xts, rns, simn, gatesb = _prep_inputs(
        hidden_states, sim_matrix, gates, temperature, experts_mask
    )
    mask, logits, _ = run_on_device(xts, rns, simn, gatesb, k)
    return mask, logits
